# revision 3
# baseline (speedup 1.0000x reference)
"""BinaryConvBNReLU Trainium2 kernel (8 NeuronCores, data-parallel over batch).

Reference computation (per nn.Module):
  bx = sign(x);  wc = clip(w, -1, 1);  alpha = mean(|wc|);  bw = sign(wc) * alpha
  out = conv2d(bx, bw, stride 1, pad 1) + x          (identity shortcut)
  out = batchnorm(out, batch stats over (B, H, W), gamma, beta, eps=1e-5)
  y = relu(out)

Strategy:
  - Batch sharded 4 images/core; weights replicated.
  - conv(sign x, sign w) on TensorE as 9 shifted bf16 matmuls per 128-channel
    chunk pair (+-1 values are exact in bf16; PSUM accumulates exact integers);
    alpha folded in at PSUM eviction: out = alpha*psum + x.
  - BN batch stats: per-core per-channel sum / sum-of-squares accumulated via
    fused accum outputs, AllReduced across the 8 cores (2 x 1KB), then
    normalize+ReLU applied on ScalarE as relu(scale*out + bias).
"""

import numpy as np

B, C, H, W = 32, 256, 56, 56
K = 3
EPS = 1e-5
N_CORES = 8
B_LOC = B // N_CORES          # 4 images per core
P = 128                       # SBUF partitions
NCH = C // P                  # 2 channel chunks
HW = H * W                    # 3136
HP, WP = H + 2, W + 2         # 58x58 zero-padded sign(x) layout
ROWS = 8                      # output rows per PSUM tile
NRT = H // ROWS               # 7 row tiles per image
NT = ROWS * W                 # 448 pixels per PSUM tile (<=512 fp32 bank)
COUNT = B * HW                # BN reduction count (global batch)

_CACHE = {}


def _build_nc():
    import concourse.bacc as bacc
    import concourse.bass_isa as bass_isa
    import concourse.mybir as mybir
    import concourse.tile as tile
    from concourse.masks import make_identity
    from contextlib import ExitStack

    f32 = mybir.dt.float32
    bf16 = mybir.dt.bfloat16
    Alu = mybir.AluOpType
    Act = mybir.ActivationFunctionType
    AxisX = mybir.AxisListType.X

    nc = bacc.Bacc(
        "TRN2", target_bir_lowering=False, debug=False, num_devices=N_CORES
    )
    x_d = nc.dram_tensor("x", [B_LOC, C, H, W], f32, kind="ExternalInput")
    w_d = nc.dram_tensor("w", [C, C, K, K], f32, kind="ExternalInput")
    g_d = nc.dram_tensor("gamma", [C], f32, kind="ExternalInput")
    be_d = nc.dram_tensor("beta", [C], f32, kind="ExternalInput")
    y_d = nc.dram_tensor("y", [B_LOC, C, H, W], f32, kind="ExternalOutput")

    with tile.TileContext(nc) as tc, ExitStack() as es:
        big = es.enter_context(tc.tile_pool(name="big", bufs=1))
        wpool = es.enter_context(tc.tile_pool(name="wpool", bufs=1))
        wst = es.enter_context(tc.tile_pool(name="wst", bufs=2))
        sgt = es.enter_context(tc.tile_pool(name="sgt", bufs=2))
        xpadp = es.enter_context(tc.tile_pool(name="xpadp", bufs=2))
        psum = es.enter_context(tc.tile_pool(name="psum", bufs=4, space="PSUM"))
        psum_sq = es.enter_context(tc.tile_pool(name="psum_sq", bufs=2, space="PSUM"))
        psum_t = es.enter_context(tc.tile_pool(name="psum_t", bufs=2, space="PSUM"))
        dram = es.enter_context(tc.tile_pool(name="dram", bufs=1, space="DRAM"))

        # Entire per-core activation tensor (x, then conv+x, then relu output)
        # stays resident in SBUF: [128, 4 img, 2 chunks, 3136 px] fp32.
        out_sb = big.tile([P, B_LOC, NCH, HW], f32, name="out_sb")
        # Transposed sign weights: [ci_local, ci_chunk, tap, co] bf16.
        wT = wpool.tile([P, NCH, K * K, C], bf16, name="wT")
        identity = wpool.tile([P, P], bf16, name="identity")
        make_identity(nc, identity)

        gamma_sb = wpool.tile([P, NCH], f32, name="gamma_sb")
        nc.sync.dma_start(gamma_sb[:], g_d.ap().rearrange("(j p) -> p j", p=P))
        beta_sb = wpool.tile([P, NCH], f32, name="beta_sb")
        nc.sync.dma_start(beta_sb[:], be_d.ap().rearrange("(j p) -> p j", p=P))

        sum_stat = wpool.tile([P, NCH, B_LOC * NRT], f32, name="sum_stat")
        sq_stat = wpool.tile([P, NCH, B_LOC * NRT], f32, name="sq_stat")
        eps_sb = wpool.tile([P, 1], f32, name="eps_sb")
        nc.vector.memset(eps_sb[:], EPS)

        # ---- weight preprocessing -------------------------------------
        w_flat = w_d.ap().rearrange("o i kh kw -> o (i kh kw)")
        a_parts = wpool.tile([P, NCH], f32, name="a_parts")
        for j in range(NCH):
            w_sb = wst.tile([P, C * K * K], f32, tag="wsb", name=f"wsb{j}")
            nc.sync.dma_start(w_sb[:], w_flat[j * P : (j + 1) * P, :])
            # clip(w, -1, 1) in place (sign unchanged, needed for alpha)
            nc.vector.tensor_scalar(w_sb[:], w_sb[:], 1.0, -1.0, Alu.min, Alu.max)
            nc.vector.tensor_reduce(
                a_parts[:, j : j + 1],
                w_sb[:],
                axis=AxisX,
                op=Alu.add,
                apply_absolute_value=True,
            )
            # sign(w) -> bf16, tap-major layout [co_local, tap, ci]
            sgn = sgt.tile([P, K * K, C], bf16, tag="sgn", name=f"sgn{j}")
            nc.scalar.activation(
                sgn[:], w_sb.rearrange("p (c t) -> p t c", t=K * K), Act.Sign
            )
            # transpose each [co,ci] 128x128 block on TensorE -> [ci, co]
            for t in range(K * K):
                for k in range(NCH):
                    pt = psum_t.tile([P, P], bf16, tag="pt", name=f"pt{j}_{t}_{k}")
                    nc.tensor.transpose(pt[:], sgn[:, t, k * P : (k + 1) * P], identity[:])
                    nc.scalar.copy(wT[:, k, t, j * P : (j + 1) * P], pt[:])

        a_sum = wpool.tile([P, 1], f32, name="a_sum")
        nc.vector.tensor_reduce(a_sum[:], a_parts[:], axis=AxisX, op=Alu.add)
        a_all = wpool.tile([P, 1], f32, name="a_all")
        nc.gpsimd.partition_all_reduce(
            a_all[:], a_sum[:], channels=P, reduce_op=bass_isa.ReduceOp.add
        )
        alpha = wpool.tile([P, 1], f32, name="alpha")
        nc.scalar.mul(alpha[:], a_all[:], 1.0 / (C * C * K * K))

        # ---- conv + shortcut + stats ----------------------------------
        x_flat = x_d.ap().rearrange("b c h w -> b c (h w)")
        y_flat = y_d.ap().rearrange("b c h w -> b c (h w)")
        for b in range(B_LOC):
            xpad = xpadp.tile([P, NCH, HP, WP], bf16, tag="xpad", name=f"xpad{b}")
            nc.gpsimd.memset(xpad[:], 0.0)
            for k in range(NCH):
                nc.sync.dma_start(out_sb[:, b, k, :], x_flat[b, k * P : (k + 1) * P, :])
                nc.scalar.activation(
                    xpad[:, k, 1 : H + 1, 1 : W + 1],
                    out_sb[:, b, k, :].rearrange("p (h w) -> p h w", h=H),
                    Act.Sign,
                )
            for j in range(NCH):
                for rt in range(NRT):
                    ps = psum.tile([P, NT], f32, tag="ps", name=f"ps{b}_{j}_{rt}")
                    mm = 0
                    for kh in range(K):
                        for kw in range(K):
                            for k in range(NCH):
                                nc.tensor.matmul(
                                    ps[:],
                                    wT[:, k, kh * K + kw, j * P : (j + 1) * P],
                                    xpad[:, k, rt * ROWS + kh : rt * ROWS + kh + ROWS, kw : kw + W],
                                    start=(mm == 0),
                                    stop=(mm == 2 * K * K - 1),
                                )
                                mm += 1
                    idx = b * NRT + rt
                    sl = out_sb[:, b, j, rt * NT : (rt + 1) * NT]
                    # out = alpha*conv + x (in place over x), accum -> per-tile sum
                    nc.vector.scalar_tensor_tensor(
                        out=sl,
                        in0=ps[:],
                        scalar=alpha[:],
                        in1=sl,
                        op0=Alu.mult,
                        op1=Alu.add,
                        accum_out=sum_stat[:, j, idx : idx + 1],
                    )
                    sq = psum_sq.tile([P, NT], f32, tag="sq", name=f"sq{b}_{j}_{rt}")
                    nc.scalar.activation(
                        sq[:], sl, Act.Square,
                        accum_out=sq_stat[:, j, idx : idx + 1],
                    )

        # ---- sync-BN stats + normalize + relu + store -----------------
        stats_loc = wpool.tile([P, NCH, 2], f32, name="stats_loc")
        for j in range(NCH):
            nc.vector.tensor_reduce(
                stats_loc[:, j, 0:1], sum_stat[:, j, :], axis=AxisX, op=Alu.add
            )
            nc.vector.tensor_reduce(
                stats_loc[:, j, 1:2], sq_stat[:, j, :], axis=AxisX, op=Alu.add
            )
            bnc_in = dram.tile([P, 2], f32, name=f"bncin{j}")
            bnc_out = dram.tile([P, 2], f32, name=f"bncout{j}", addr_space="Shared")
            nc.gpsimd.dma_start(bnc_in[:], stats_loc[:, j, :])
            nc.gpsimd.collective_compute(
                "AllReduce",
                Alu.add,
                replica_groups=[list(range(N_CORES))],
                ins=[bnc_in.opt()],
                outs=[bnc_out.opt()],
            )
            glob = wpool.tile([P, 2], f32, name=f"glob{j}")
            nc.gpsimd.dma_start(glob[:], bnc_out[:])

            mean = wpool.tile([P, 1], f32, name=f"mean{j}")
            nc.scalar.mul(mean[:], glob[:, 0:1], 1.0 / COUNT)
            ex2 = wpool.tile([P, 1], f32, name=f"ex2{j}")
            nc.scalar.mul(ex2[:], glob[:, 1:2], 1.0 / COUNT)
            msq = wpool.tile([P, 1], f32, name=f"msq{j}")
            nc.vector.tensor_mul(msq[:], mean[:], mean[:])
            var = wpool.tile([P, 1], f32, name=f"var{j}")
            nc.vector.tensor_sub(var[:], ex2[:], msq[:])
            sd = wpool.tile([P, 1], f32, name=f"sd{j}")
            nc.scalar.activation(sd[:], var[:], Act.Sqrt, bias=eps_sb[:])
            rinv = wpool.tile([P, 1], f32, name=f"rinv{j}")
            nc.vector.reciprocal(rinv[:], sd[:])
            scl = wpool.tile([P, 1], f32, name=f"scl{j}")
            nc.vector.tensor_mul(scl[:], rinv[:], gamma_sb[:, j : j + 1])
            mscl = wpool.tile([P, 1], f32, name=f"mscl{j}")
            nc.vector.tensor_mul(mscl[:], mean[:], scl[:])
            bia = wpool.tile([P, 1], f32, name=f"bia{j}")
            nc.vector.tensor_sub(bia[:], beta_sb[:, j : j + 1], mscl[:])

            for b in range(B_LOC):
                sl = out_sb[:, b, j, :]
                nc.scalar.activation(sl, sl, Act.Relu, bias=bia[:], scale=scl[:])
                nc.sync.dma_start(y_flat[b, j * P : (j + 1) * P, :], sl)

    nc.compile()
    return nc


def _get_nc():
    if "nc" not in _CACHE:
        _CACHE["nc"] = _build_nc()
    return _CACHE["nc"]


def _run(in_maps, trace=False, tmpdir=None):
    import concourse.bass_utils as bass_utils

    nc = _get_nc()
    return bass_utils.run_bass_kernel_spmd(
        nc, in_maps, core_ids=list(range(N_CORES)), trace=trace, tmpdir=tmpdir
    )


def _make_in_maps(x, w, gamma, beta):
    x = np.ascontiguousarray(np.asarray(x), dtype=np.float32)
    w = np.ascontiguousarray(np.asarray(w), dtype=np.float32)
    gamma = np.ascontiguousarray(np.asarray(gamma), dtype=np.float32)
    beta = np.ascontiguousarray(np.asarray(beta), dtype=np.float32)
    assert x.shape == (B, C, H, W)
    xs = np.split(x, N_CORES, axis=0)
    return [
        {"x": xs[i], "w": w, "gamma": gamma, "beta": beta} for i in range(N_CORES)
    ]


def kernel(x, w, gamma, beta):
    in_maps = _make_in_maps(x, w, gamma, beta)
    res = _run(in_maps, trace=False)
    return np.concatenate([r["y"] for r in res.results], axis=0)


# ---- profiling helpers (used by test.py only) -------------------------

def _install_ntff_hook_shim():
    """bass_utils wants antenv.axon_hooks for NTFF tracing under axon; shim it."""
    import sys
    import types

    import antenv

    if "antenv.axon_hooks" in sys.modules:
        return
    mod = types.ModuleType("antenv.axon_hooks")
    mod._hook = None
    mod.set_axon_ntff_profile_hook = lambda h: setattr(mod, "_hook", h)
    mod.get_axon_ntff_profile_hook = lambda: mod._hook
    sys.modules["antenv.axon_hooks"] = mod
    antenv.axon_hooks = mod

    from trn_agent_boot.trn_boot import _ntff_profile_via_ctypes

    mod.set_axon_ntff_profile_hook(
        _ntff_profile_via_ctypes("/opt/axon/libaxon_pjrt.so")
    )


def kernel_traced(x, w, gamma, beta, tmpdir=None):
    """Run once with NTFF profiling; returns (y_full, exec_time_ns, trace_path)."""
    import concourse.bass_utils as bass_utils

    _install_ntff_hook_shim()
    bass_utils.upload_artifacts = lambda d: "local://disabled"
    in_maps = _make_in_maps(x, w, gamma, beta)
    res = _run(in_maps, trace=True, tmpdir=tmpdir)
    y = np.concatenate([r["y"] for r in res.results], axis=0)
    trace_path = (
        res.instructions_and_trace[1] if res.instructions_and_trace else None
    )
    return y, res.exec_time_ns, trace_path


# revision 6
# speedup vs baseline: 1.0297x; 1.0297x over previous
"""BinaryConvBNReLU Trainium2 kernel (8 NeuronCores, data-parallel over batch).

Reference computation (per nn.Module):
  bx = sign(x);  wc = clip(w, -1, 1);  alpha = mean(|wc|);  bw = sign(wc) * alpha
  out = conv2d(bx, bw, stride 1, pad 1) + x          (identity shortcut)
  out = batchnorm(out, batch stats over (B, H, W), gamma, beta, eps=1e-5)
  y = relu(out)

Strategy:
  - Batch sharded 4 images/core; weights replicated.
  - conv(sign x, sign w) on TensorE as 9 shifted bf16 matmuls per 128-channel
    chunk pair (+-1 values are exact in bf16; PSUM accumulates exact integers);
    alpha folded in at PSUM eviction: out = alpha*psum + x.
  - BN batch stats: per-core per-channel sum / sum-of-squares accumulated via
    fused accum outputs, AllReduced across the 8 cores (2 x 1KB), then
    normalize+ReLU applied on ScalarE as relu(scale*out + bias).
"""

import numpy as np

B, C, H, W = 32, 256, 56, 56
K = 3
EPS = 1e-5
N_CORES = 8
B_LOC = B // N_CORES          # 4 images per core
P = 128                       # SBUF partitions
NCH = C // P                  # 2 channel chunks
HW = H * W                    # 3136
HP, WP = H + 2, W + 2         # 58x58 zero-padded sign(x) layout
ROWS = 8                      # output rows per PSUM tile
NRT = H // ROWS               # 7 row tiles per image
NT = ROWS * W                 # 448 pixels per PSUM tile (<=512 fp32 bank)
COUNT = B * HW                # BN reduction count (global batch)

_CACHE = {}


def _build_nc():
    import concourse.bacc as bacc
    import concourse.bass_isa as bass_isa
    import concourse.mybir as mybir
    import concourse.tile as tile
    from concourse.masks import make_identity
    from contextlib import ExitStack

    f32 = mybir.dt.float32
    bf16 = mybir.dt.bfloat16
    Alu = mybir.AluOpType
    Act = mybir.ActivationFunctionType
    AxisX = mybir.AxisListType.X

    nc = bacc.Bacc(
        "TRN2", target_bir_lowering=False, debug=False, num_devices=N_CORES
    )
    x_d = nc.dram_tensor("x", [B_LOC, C, H, W], f32, kind="ExternalInput")
    w_d = nc.dram_tensor("w", [C, C, K, K], f32, kind="ExternalInput")
    g_d = nc.dram_tensor("gamma", [C], f32, kind="ExternalInput")
    be_d = nc.dram_tensor("beta", [C], f32, kind="ExternalInput")
    y_d = nc.dram_tensor("y", [B_LOC, C, H, W], f32, kind="ExternalOutput")

    with tile.TileContext(nc) as tc, ExitStack() as es:
        big = es.enter_context(tc.tile_pool(name="big", bufs=1))
        wpool = es.enter_context(tc.tile_pool(name="wpool", bufs=1))
        wst = es.enter_context(tc.tile_pool(name="wst", bufs=1))
        sgt = es.enter_context(tc.tile_pool(name="sgt", bufs=2))
        xpadp = es.enter_context(tc.tile_pool(name="xpadp", bufs=B_LOC))
        psum = es.enter_context(tc.tile_pool(name="psum", bufs=4, space="PSUM"))
        psum_sq = es.enter_context(tc.tile_pool(name="psum_sq", bufs=2, space="PSUM"))
        psum_t = es.enter_context(tc.tile_pool(name="psum_t", bufs=2, space="PSUM"))
        dram = es.enter_context(tc.tile_pool(name="dram", bufs=1, space="DRAM"))

        # Entire per-core activation tensor (x, then conv+x, then relu output)
        # stays resident in SBUF: [128, 4 img, 2 chunks, 3136 px] fp32.
        out_sb = big.tile([P, B_LOC, NCH, HW], f32, name="out_sb")
        # Transposed sign weights: [ci_local, ci_chunk, tap, co] bf16.
        wT = wpool.tile([P, NCH, K * K, C], bf16, name="wT")
        identity = wpool.tile([P, P], bf16, name="identity")
        make_identity(nc, identity)

        gamma_sb = wpool.tile([P, NCH], f32, name="gamma_sb")
        nc.sync.dma_start(gamma_sb[:], g_d.ap().rearrange("(j p) -> p j", p=P))
        beta_sb = wpool.tile([P, NCH], f32, name="beta_sb")
        nc.sync.dma_start(beta_sb[:], be_d.ap().rearrange("(j p) -> p j", p=P))

        sum_stat = wpool.tile([P, NCH, B_LOC * NRT], f32, name="sum_stat")
        sq_stat = wpool.tile([P, NCH, B_LOC * NRT], f32, name="sq_stat")
        eps_sb = wpool.tile([P, 1], f32, name="eps_sb")
        nc.vector.memset(eps_sb[:], EPS)

        # ---- weight preprocessing -------------------------------------
        w_flat = w_d.ap().rearrange("o i kh kw -> o (i kh kw)")
        a_parts = wpool.tile([P, NCH], f32, name="a_parts")
        for j in range(NCH):
            w_sb = wst.tile([P, C * K * K], f32, tag="wsb", name=f"wsb{j}")
            nc.sync.dma_start(w_sb[:], w_flat[j * P : (j + 1) * P, :])
            w_taps = w_sb.rearrange("p (c t) -> p t c", t=K * K)
            # sign(w) -> bf16, tap-major layout [co_local, tap, ci]; one ACT op
            # per tap so TensorE transposes can start after the first tap.
            sgn = sgt.tile([P, K * K, C], bf16, tag="sgn", name=f"sgn{j}")
            for t in range(K * K):
                nc.scalar.activation(sgn[:, t, :], w_taps[:, t, :], Act.Sign)
                # transpose each [co,ci] 128x128 block on TensorE -> [ci, co]
                for k in range(NCH):
                    pt = psum_t.tile([P, P], bf16, tag="pt", name=f"pt{j}_{t}_{k}")
                    nc.tensor.transpose(pt[:], sgn[:, t, k * P : (k + 1) * P], identity[:])
                    nc.scalar.copy(wT[:, k, t, j * P : (j + 1) * P], pt[:])
            # clip(w, -1, 1) in place (sign unchanged; needed for alpha only)
            nc.vector.tensor_scalar(w_sb[:], w_sb[:], 1.0, -1.0, Alu.min, Alu.max)
            nc.vector.tensor_reduce(
                a_parts[:, j : j + 1],
                w_sb[:],
                axis=AxisX,
                op=Alu.add,
                apply_absolute_value=True,
            )

        a_sum = wpool.tile([P, 1], f32, name="a_sum")
        nc.vector.tensor_reduce(a_sum[:], a_parts[:], axis=AxisX, op=Alu.add)
        a_all = wpool.tile([P, 1], f32, name="a_all")
        nc.gpsimd.partition_all_reduce(
            a_all[:], a_sum[:], channels=P, reduce_op=bass_isa.ReduceOp.add
        )
        alpha = wpool.tile([P, 1], f32, name="alpha")
        nc.scalar.mul(alpha[:], a_all[:], 1.0 / (C * C * K * K))

        # ---- conv + shortcut + stats ----------------------------------
        # j-outer: all images' chunk-0 outputs (and their BN stats) finish at
        # the halfway point, so chunk 0's AllReduce + normalize + store fully
        # overlap chunk 1's conv.
        x_flat = x_d.ap().rearrange("b c h w -> b c (h w)")
        y_flat = y_d.ap().rearrange("b c h w -> b c (h w)")
        xpads = []
        stats_loc = wpool.tile([P, NCH, 2], f32, name="stats_loc")
        for b in range(B_LOC):
            xpad = xpadp.tile([P, NCH, HP, WP], bf16, tag="xpad", name=f"xpad{b}")
            xpads.append(xpad)
        for j in range(NCH):
            for b in range(B_LOC):
                xpad = xpads[b]
                if j == 0:
                    nc.gpsimd.memset(xpad[:], 0.0)
                    for k in range(NCH):
                        nc.sync.dma_start(
                            out_sb[:, b, k, :], x_flat[b, k * P : (k + 1) * P, :]
                        )
                        nc.scalar.activation(
                            xpad[:, k, 1 : H + 1, 1 : W + 1],
                            out_sb[:, b, k, :].rearrange("p (h w) -> p h w", h=H),
                            Act.Sign,
                        )
                for rt in range(NRT):
                    ps = psum.tile([P, NT], f32, tag="ps", name=f"ps{b}_{j}_{rt}")
                    mm = 0
                    for kh in range(K):
                        for kw in range(K):
                            for k in range(NCH):
                                nc.tensor.matmul(
                                    ps[:],
                                    wT[:, k, kh * K + kw, j * P : (j + 1) * P],
                                    xpad[:, k, rt * ROWS + kh : rt * ROWS + kh + ROWS, kw : kw + W],
                                    start=(mm == 0),
                                    stop=(mm == 2 * K * K - 1),
                                )
                                mm += 1
                    idx = b * NRT + rt
                    sl = out_sb[:, b, j, rt * NT : (rt + 1) * NT]
                    # out = alpha*conv + x (in place over x), accum -> per-tile sum
                    nc.vector.scalar_tensor_tensor(
                        out=sl,
                        in0=ps[:],
                        scalar=alpha[:],
                        in1=sl,
                        op0=Alu.mult,
                        op1=Alu.add,
                        accum_out=sum_stat[:, j, idx : idx + 1],
                    )
                    sq = psum_sq.tile([P, NT], f32, tag="sq", name=f"sq{b}_{j}_{rt}")
                    nc.scalar.activation(
                        sq[:], sl, Act.Square,
                        accum_out=sq_stat[:, j, idx : idx + 1],
                    )

            # ---- sync-BN stats + normalize + relu + store for chunk j ----
            nc.vector.tensor_reduce(
                stats_loc[:, j, 0:1], sum_stat[:, j, :], axis=AxisX, op=Alu.add
            )
            nc.vector.tensor_reduce(
                stats_loc[:, j, 1:2], sq_stat[:, j, :], axis=AxisX, op=Alu.add
            )
            bnc_in = dram.tile([P, 2], f32, name=f"bncin{j}")
            bnc_out = dram.tile([P, 2], f32, name=f"bncout{j}", addr_space="Shared")
            nc.gpsimd.dma_start(bnc_in[:], stats_loc[:, j, :])
            nc.gpsimd.collective_compute(
                "AllReduce",
                Alu.add,
                replica_groups=[list(range(N_CORES))],
                ins=[bnc_in.opt()],
                outs=[bnc_out.opt()],
            )
            glob = wpool.tile([P, 2], f32, name=f"glob{j}")
            nc.gpsimd.dma_start(glob[:], bnc_out[:])

            mean = wpool.tile([P, 1], f32, name=f"mean{j}")
            nc.scalar.mul(mean[:], glob[:, 0:1], 1.0 / COUNT)
            ex2 = wpool.tile([P, 1], f32, name=f"ex2{j}")
            nc.scalar.mul(ex2[:], glob[:, 1:2], 1.0 / COUNT)
            msq = wpool.tile([P, 1], f32, name=f"msq{j}")
            nc.vector.tensor_mul(msq[:], mean[:], mean[:])
            var = wpool.tile([P, 1], f32, name=f"var{j}")
            nc.vector.tensor_sub(var[:], ex2[:], msq[:])
            sd = wpool.tile([P, 1], f32, name=f"sd{j}")
            nc.scalar.activation(sd[:], var[:], Act.Sqrt, bias=eps_sb[:])
            rinv = wpool.tile([P, 1], f32, name=f"rinv{j}")
            nc.vector.reciprocal(rinv[:], sd[:])
            scl = wpool.tile([P, 1], f32, name=f"scl{j}")
            nc.vector.tensor_mul(scl[:], rinv[:], gamma_sb[:, j : j + 1])
            mscl = wpool.tile([P, 1], f32, name=f"mscl{j}")
            nc.vector.tensor_mul(mscl[:], mean[:], scl[:])
            bia = wpool.tile([P, 1], f32, name=f"bia{j}")
            nc.vector.tensor_sub(bia[:], beta_sb[:, j : j + 1], mscl[:])

            for b in range(B_LOC):
                sl = out_sb[:, b, j, :]
                nc.scalar.activation(sl, sl, Act.Relu, bias=bia[:], scale=scl[:])
                nc.sync.dma_start(y_flat[b, j * P : (j + 1) * P, :], sl)

    nc.compile()
    return nc


def _get_nc():
    if "nc" not in _CACHE:
        _CACHE["nc"] = _build_nc()
    return _CACHE["nc"]


def _run(in_maps, trace=False, tmpdir=None):
    import concourse.bass_utils as bass_utils

    nc = _get_nc()
    return bass_utils.run_bass_kernel_spmd(
        nc, in_maps, core_ids=list(range(N_CORES)), trace=trace, tmpdir=tmpdir
    )


def _make_in_maps(x, w, gamma, beta):
    x = np.ascontiguousarray(np.asarray(x), dtype=np.float32)
    w = np.ascontiguousarray(np.asarray(w), dtype=np.float32)
    gamma = np.ascontiguousarray(np.asarray(gamma), dtype=np.float32)
    beta = np.ascontiguousarray(np.asarray(beta), dtype=np.float32)
    assert x.shape == (B, C, H, W)
    xs = np.split(x, N_CORES, axis=0)
    return [
        {"x": xs[i], "w": w, "gamma": gamma, "beta": beta} for i in range(N_CORES)
    ]


def kernel(x, w, gamma, beta):
    in_maps = _make_in_maps(x, w, gamma, beta)
    res = _run(in_maps, trace=False)
    return np.concatenate([r["y"] for r in res.results], axis=0)


# ---- profiling helpers (used by test.py only) -------------------------

def _install_ntff_hook_shim():
    """bass_utils wants antenv.axon_hooks for NTFF tracing under axon; shim it."""
    import sys
    import types

    import antenv

    if "antenv.axon_hooks" in sys.modules:
        return
    mod = types.ModuleType("antenv.axon_hooks")
    mod._hook = None
    mod.set_axon_ntff_profile_hook = lambda h: setattr(mod, "_hook", h)
    mod.get_axon_ntff_profile_hook = lambda: mod._hook
    sys.modules["antenv.axon_hooks"] = mod
    antenv.axon_hooks = mod

    from trn_agent_boot.trn_boot import _ntff_profile_via_ctypes

    mod.set_axon_ntff_profile_hook(
        _ntff_profile_via_ctypes("/opt/axon/libaxon_pjrt.so")
    )


def kernel_traced(x, w, gamma, beta, tmpdir=None):
    """Run once with NTFF profiling; returns (y_full, exec_time_ns, trace_path)."""
    import concourse.bass_utils as bass_utils

    _install_ntff_hook_shim()
    bass_utils.upload_artifacts = lambda d: "local://disabled"
    in_maps = _make_in_maps(x, w, gamma, beta)
    res = _run(in_maps, trace=True, tmpdir=tmpdir)
    y = np.concatenate([r["y"] for r in res.results], axis=0)
    trace_path = (
        res.instructions_and_trace[1] if res.instructions_and_trace else None
    )
    return y, res.exec_time_ns, trace_path
